# revision 11
# baseline (speedup 1.0000x reference)
"""FLDAttention Trainium2 kernel.

B=16, P=128, S=4096, E=512, H=8, D=64, LATENT=128.
Sharding: data-parallel over B across 8 cores (2 batches/core), no collectives.

Per-batch on-device pipeline (all matmul operands fp16, fp32 PSUM accum):
  1. K,V loaded fp32 -> cast fp16 (ACT) -> DMA-transposed (X-bar) to put E on
     partitions.
  2. KpT = wk^T @ K^T  (E_out x S, fp16 in SBUF);  Vp = V @ wv (S x E_out).
  3. Per (head, s-chunk of 128): scoresT = KpT_h_chunk^T... PE matmul
     (lhsT=KpT chunk (64,128), rhs=QpT_h (64,128)) -> PSUM (s-chunk, P).
     exp on ACT with scale=1/8 and per-partition bias folding the key-padding
     mask (+ constant -4 shift for fp16 range safety); output fp16.
  4. AV: lhsT=expT chunk, rhs=Vp chunk head slice -> C (P, E) accumulated in
     one PSUM bank; denominator l = expT^T @ ones accumulated likewise.
  5. C -> fp16, PE-transpose, out = C @ wo, multiply by 1/l (per-partition),
     DMA out. Bias adds (bq/bk/bv/bo) are emitted only when nonzero.
"""

import os
import sys

import numpy as np

if "/opt/trn_rl_repo" not in sys.path:
    sys.path.insert(0, "/opt/trn_rl_repo")

B, P, S, E = 16, 128, 4096, 512
H = 8
D = E // H
LATENT = 128
N_CORES = 8
BPC = B // N_CORES  # batches per core

SC = 128          # attention S-chunk (scoresT partition dim)
NSC = S // SC     # 32
S8 = 512          # projection S-chunk
NS8 = S // S8     # 8
NE = E // 128     # 4 E-chunks

MASK_ON = -4.0    # constant shift keeps exp() in comfortable fp16 range
MASK_OFF = -34.0  # exp(-34+s/8) ~ 1e-14 -> flushes to 0 in fp16


def build_module(n_batches, has_bq, has_bk, has_bv, has_bo):
    import concourse.bass as bass
    import concourse.mybir as mybir
    import concourse.tile as tile
    from concourse import bacc
    from concourse.masks import make_identity
    from contextlib import ExitStack

    f32 = mybir.dt.float32
    f16 = mybir.dt.float16
    AF = mybir.ActivationFunctionType

    nc = bacc.Bacc("TRN2", target_bir_lowering=False, debug=False,
                   enable_asserts=False, num_devices=N_CORES)

    q_d = nc.dram_tensor("q", [n_batches, P, E], f32, kind="ExternalInput").ap()
    k_d = nc.dram_tensor("k", [n_batches, S, E], f32, kind="ExternalInput").ap()
    v_d = nc.dram_tensor("v", [n_batches, S, E], f32, kind="ExternalInput").ap()
    mb_d = nc.dram_tensor("maskbias", [n_batches, SC, NSC], f32,
                          kind="ExternalInput").ap()
    wq_d = nc.dram_tensor("wq", [E, E], f32, kind="ExternalInput").ap()
    wk_d = nc.dram_tensor("wk", [E, E], f32, kind="ExternalInput").ap()
    wv_d = nc.dram_tensor("wv", [E, E], f32, kind="ExternalInput").ap()
    wo_d = nc.dram_tensor("wo", [E, LATENT], f32, kind="ExternalInput").ap()
    bq_d = nc.dram_tensor("bq", [E], f32, kind="ExternalInput").ap() if has_bq else None
    bk_d = nc.dram_tensor("bk", [E], f32, kind="ExternalInput").ap() if has_bk else None
    bv_d = nc.dram_tensor("bv", [E], f32, kind="ExternalInput").ap() if has_bv else None
    bo_d = nc.dram_tensor("bo", [LATENT], f32, kind="ExternalInput").ap() if has_bo else None
    out_d = nc.dram_tensor("out", [n_batches, P, LATENT], f32,
                           kind="ExternalOutput").ap()

    with tile.TileContext(nc) as tc, ExitStack() as ctx:
        const_pool = ctx.enter_context(tc.tile_pool(name="const", bufs=1))
        wstage_pool = ctx.enter_context(tc.tile_pool(name="wstage", bufs=2))
        kv_raw_pool = ctx.enter_context(tc.tile_pool(name="kvraw", bufs=3))
        kv_c16_pool = ctx.enter_context(tc.tile_pool(name="kvc16", bufs=3))
        kt_pool = ctx.enter_context(tc.tile_pool(name="ktw", bufs=2))
        vt_pool = ctx.enter_context(tc.tile_pool(name="vtw", bufs=2))
        kpt_pool = ctx.enter_context(tc.tile_pool(name="kpt", bufs=2))
        vp_pool = ctx.enter_context(tc.tile_pool(name="vp", bufs=2))
        qp_pool = ctx.enter_context(tc.tile_pool(name="qp", bufs=2))
        mb_pool = ctx.enter_context(tc.tile_pool(name="mb", bufs=2))
        ex_pool = ctx.enter_context(tc.tile_pool(name="ex", bufs=6))
        fin_pool = ctx.enter_context(tc.tile_pool(name="fin", bufs=2))

        ps_proj_pool = ctx.enter_context(
            tc.tile_pool(name="psproj", bufs=2, space="PSUM"))
        ps_sc_pool = ctx.enter_context(
            tc.tile_pool(name="pssc", bufs=2, space="PSUM"))
        ps_c_pool = ctx.enter_context(
            tc.tile_pool(name="psc", bufs=1, space="PSUM"))
        ps_fin_pool = ctx.enter_context(
            tc.tile_pool(name="psfin", bufs=2, space="PSUM"))

        # ---- constants & weights (once) ----
        ident = const_pool.tile([128, 128], f16, name="ident")
        make_identity(nc, ident[:])

        def load_weight_f16(w_ap, ncols, name):
            tiles = []
            for j in range(NE):
                raw = wstage_pool.tile([128, ncols], f32, tag="wraw")
                nc.sync.dma_start(raw[:], w_ap[j * 128:(j + 1) * 128, :])
                w16 = const_pool.tile([128, ncols], f16, name=f"{name}{j}")
                nc.scalar.copy(w16[:], raw[:])
                tiles.append(w16)
            return tiles

        wq_h = load_weight_f16(wq_d, E, "wq")
        wk_h = load_weight_f16(wk_d, E, "wk")
        wv_h = load_weight_f16(wv_d, E, "wv")
        wo_h = load_weight_f16(wo_d, LATENT, "wo")

        ones_row = None
        if has_bv or has_bo:
            ones_row = const_pool.tile([1, 128], f16, name="ones_row")
            nc.vector.memset(ones_row[:], 1.0)

        def load_vec_f16(b_ap, n, name):
            raw = wstage_pool.tile([1, n], f32, tag="braw")
            nc.sync.dma_start(raw[:], b_ap[None, :])
            v16 = const_pool.tile([1, n], f16, name=name)
            nc.scalar.copy(v16[:], raw[:])
            return v16

        bv_row = load_vec_f16(bv_d, E, "bv_row") if has_bv else None
        bo_row = load_vec_f16(bo_d, LATENT, "bo_row") if has_bo else None

        def load_bias_cols(b_ap, name):
            # (E,) -> 4 tiles (128,1) f32 for per-partition activation bias
            tiles = []
            for j in range(NE):
                t = const_pool.tile([128, 1], f32, name=f"{name}{j}")
                nc.sync.dma_start(t[:], b_ap[j * 128:(j + 1) * 128, None])
                tiles.append(t)
            return tiles

        bq_cols = load_bias_cols(bq_d, "bq") if has_bq else None
        bk_cols = load_bias_cols(bk_d, "bk") if has_bk else None

        for b in range(n_batches):
            # ---- mask bias (128, 32) f32: [p, c] = bias for s = c*128+p ----
            mbias = mb_pool.tile([128, NSC], f32, tag="mb")
            nc.sync.dma_start(mbias[:], mb_d[b])

            # ---- Q path: QT then QpT (E_out on partitions) ----
            qraw = kv_raw_pool.tile([128, E], f32, tag="qraw")
            nc.sync.dma_start(qraw[:], q_d[b])
            qc16 = kv_c16_pool.tile([128, E], f16, tag="qc16")
            nc.scalar.copy(qc16[:], qraw[:])
            qt = []
            for j in range(NE):
                qt_j = qp_pool.tile([128, 128], f16, tag=f"qt{j}")
                nc.sync.dma_start_transpose(qt_j[:], qc16[:, j * 128:(j + 1) * 128])
                qt.append(qt_j)
            qpt = []
            for m in range(NE):
                ps = ps_fin_pool.tile([128, 128], f32, tag="fin")
                for j in range(NE):
                    nc.tensor.matmul(ps[:], wq_h[j][:, m * 128:(m + 1) * 128],
                                     qt[j][:], start=(j == 0), stop=(j == NE - 1))
                t = qp_pool.tile([128, 128], f16, tag=f"qpt{m}")
                if has_bq:
                    nc.scalar.activation(t[:], ps[:], AF.Identity,
                                         bias=bq_cols[m][:], scale=1.0)
                else:
                    nc.vector.tensor_copy(t[:], ps[:])
                qpt.append(t)

            # ---- K/V transpose + projections ----
            kpt = [kpt_pool.tile([128, S], f16, tag=f"kpt{m}", name=f"kpt{b}_{m}")
                   for m in range(NE)]
            # per head: 64 Vp columns + 1 ones column (gives the per-head
            # softmax denominator for free in the AV matmul)
            vp = [vp_pool.tile([128, H, D + 1], f16, tag=f"vp{c}",
                               name=f"vp{b}_{c}")
                  for c in range(NSC)]

            for s8 in range(NS8):
                ktw = [kt_pool.tile([128, S8], f16, tag=f"kt{j}", name=f"kt{b}_{s8}_{j}")
                       for j in range(NE)]
                vtw = [vt_pool.tile([128, S8], f16, tag=f"vt{j}", name=f"vt{b}_{s8}_{j}")
                       for j in range(NE)]
                for i in range(4):
                    s0 = s8 * S8 + i * 128
                    kraw = kv_raw_pool.tile([128, E], f32, tag="kraw")
                    nc.sync.dma_start(kraw[:], k_d[b, s0:s0 + 128, :])
                    kc16 = kv_c16_pool.tile([128, E], f16, tag="kc16")
                    nc.scalar.copy(kc16[:], kraw[:])
                    vraw = kv_raw_pool.tile([128, E], f32, tag="vraw")
                    nc.sync.dma_start(vraw[:], v_d[b, s0:s0 + 128, :])
                    vc16 = kv_c16_pool.tile([128, E], f16, tag="vc16")
                    nc.vector.tensor_copy(vc16[:], vraw[:])
                    for j in range(NE):
                        nc.sync.dma_start_transpose(
                            ktw[j][:, i * 128:(i + 1) * 128],
                            kc16[:, j * 128:(j + 1) * 128])
                        nc.sync.dma_start_transpose(
                            vtw[j][:, i * 128:(i + 1) * 128],
                            vc16[:, j * 128:(j + 1) * 128])

                # KpT chunk: (E_out 128-chunk m) x (S8 cols)
                for m in range(NE):
                    ps = ps_proj_pool.tile([128, S8], f32, tag="proj")
                    for j in range(NE):
                        nc.tensor.matmul(ps[:], wk_h[j][:, m * 128:(m + 1) * 128],
                                         ktw[j][:], start=(j == 0),
                                         stop=(j == NE - 1))
                    dst = kpt[m][:, s8 * S8:(s8 + 1) * S8]
                    if has_bk:
                        nc.scalar.activation(dst, ps[:], AF.Identity,
                                             bias=bk_cols[m][:], scale=1.0)
                    else:
                        nc.vector.tensor_copy(dst, ps[:])

                # Vp chunks: (S 128-chunk) x (E_out 512)
                for i in range(4):
                    c = s8 * 4 + i
                    ps = ps_proj_pool.tile([128, E], f32, tag="proj")
                    for j in range(NE):
                        nc.tensor.matmul(ps[:], vtw[j][:, i * 128:(i + 1) * 128],
                                         wv_h[j][:], start=(j == 0),
                                         stop=(j == NE - 1 and not has_bv))
                    if has_bv:
                        nc.tensor.matmul(ps[:], ones_row[:], bv_row[:],
                                         start=False, stop=True)
                    nc.vector.tensor_copy(
                        vp[c][:, :, 0:D],
                        ps[:].rearrange("p (h x) -> p h x", h=H))
                    nc.vector.memset(vp[c][:, :, D:D + 1], 1.0)

            # ---- attention ----
            # C~ spans two PSUM banks: 4 heads x (64 AV cols + 1 denom col).
            c_ps = [ps_c_pool.tile([128, 4, D + 1], f32, tag=f"c{i}",
                                   name=f"c{b}_{i}")
                    for i in range(2)]
            for c in range(NSC):
                for h in range(H):
                    t, off = h // 2, (h % 2) * 64
                    s_ps = ps_sc_pool.tile([128, 128], f32, tag="s")
                    nc.tensor.matmul(
                        s_ps[:],
                        kpt[t][off:off + 64, c * SC:(c + 1) * SC],
                        qpt[t][off:off + 64, :],
                        start=True, stop=True)
                    ex = ex_pool.tile([128, 128], f16, tag="ex")
                    nc.scalar.activation(ex[:], s_ps[:], AF.Exp,
                                         bias=mbias[:, c:c + 1], scale=0.125)
                    # One start/stop per PSUM bank (zero region), not per
                    # head: start marks the whole 2KB region pending-zero,
                    # so each head's first write overwrites, rest accumulate.
                    nc.tensor.matmul(c_ps[h // 4][:, h % 4, :], ex[:],
                                     vp[c][:, h, :],
                                     start=(c == 0 and h % 4 == 0),
                                     stop=(c == NSC - 1 and h % 4 == 3))

            # ---- normalize per head, then output projection ----
            rl_sb = fin_pool.tile([128, H], f32, tag="rl_sb")
            for h in range(H):
                nc.vector.reciprocal(rl_sb[:, h:h + 1],
                                     c_ps[h // 4][:, h % 4, D:D + 1])
            csb = fin_pool.tile([128, E], f16, tag="csb")
            for h in range(H):
                nc.vector.tensor_scalar_mul(csb[:, h * D:(h + 1) * D],
                                            c_ps[h // 4][:, h % 4, 0:D],
                                            rl_sb[:, h:h + 1])
            ct = []
            for j in range(NE):
                ct_ps = ps_fin_pool.tile([128, 128], f16, tag="fin")
                nc.tensor.transpose(ct_ps[:], csb[:, j * 128:(j + 1) * 128],
                                    ident[:])
                ct_j = fin_pool.tile([128, 128], f16, tag=f"ct{j}")
                nc.vector.tensor_copy(ct_j[:], ct_ps[:])
                ct.append(ct_j)
            o_ps = ps_fin_pool.tile([128, LATENT], f32, tag="fin")
            for j in range(NE):
                nc.tensor.matmul(o_ps[:], ct[j][:], wo_h[j][:],
                                 start=(j == 0), stop=(j == NE - 1 and not has_bo))
            if has_bo:
                nc.tensor.matmul(o_ps[:], ones_row[:], bo_row[:],
                                 start=False, stop=True)
            osb = fin_pool.tile([128, LATENT], f32, tag="osb")
            nc.vector.tensor_copy(osb[:], o_ps[:])
            nc.sync.dma_start(out_d[b], osb[:])

    nc.compile()
    return nc


_module_cache = {}


def _get_module(n_batches, flags):
    key = (n_batches, flags)
    if key not in _module_cache:
        _module_cache[key] = build_module(n_batches, *flags)
    return _module_cache[key]


def make_maskbias(mask):
    # (B, S) bool -> (B, SC, NSC) f32 with [b, p, c] = bias for s = c*128+p
    mb = np.where(mask, MASK_ON, MASK_OFF).astype(np.float32)
    return np.ascontiguousarray(mb.reshape(-1, NSC, SC).transpose(0, 2, 1))


def make_in_maps(Q, K, V, mask, wq, bq, wk, bk, wv, bv, wo, bo, n_cores, bpc):
    flags = (bool(np.any(bq)), bool(np.any(bk)),
             bool(np.any(bv)), bool(np.any(bo)))
    mb = make_maskbias(np.asarray(mask))
    f = np.ascontiguousarray
    in_maps = []
    for i in range(n_cores):
        sl = slice(i * bpc, (i + 1) * bpc)
        m = {"q": f(Q[sl]), "k": f(K[sl]), "v": f(V[sl]), "maskbias": f(mb[sl]),
             "wq": f(wq), "wk": f(wk), "wv": f(wv), "wo": f(wo)}
        if flags[0]:
            m["bq"] = f(bq)
        if flags[1]:
            m["bk"] = f(bk)
        if flags[2]:
            m["bv"] = f(bv)
        if flags[3]:
            m["bo"] = f(bo)
        in_maps.append(m)
    return in_maps, flags


def kernel(**inputs):
    from concourse.bass_utils import run_bass_kernel_spmd

    args = {k: np.asarray(v) for k, v in inputs.items()}
    in_maps, flags = make_in_maps(
        args["Q"].astype(np.float32), args["K"].astype(np.float32),
        args["V"].astype(np.float32), args["mask"],
        args["wq"].astype(np.float32), args["bq"].astype(np.float32),
        args["wk"].astype(np.float32), args["bk"].astype(np.float32),
        args["wv"].astype(np.float32), args["bv"].astype(np.float32),
        args["wo"].astype(np.float32), args["bo"].astype(np.float32),
        N_CORES, BPC)
    nc = _get_module(BPC, flags)
    res = run_bass_kernel_spmd(nc, in_maps, core_ids=list(range(N_CORES)))
    kernel.last_results = res
    if res.exec_time_ns is not None:
        print(f"HW exec time: {res.exec_time_ns} ns")
    out = np.concatenate([r["out"] for r in res.results], axis=0)
    return out.astype(np.float32)


# revision 18
# speedup vs baseline: 2.5847x; 2.5847x over previous
"""FLDAttention Trainium2 kernel.

B=16, P=128, S=4096, E=512, H=8, D=64, LATENT=128.
Sharding: data-parallel over B across 8 cores (2 batches/core), no collectives.

Per-batch on-device pipeline (all matmul operands fp16, fp32 PSUM accum):
  1. K,V loaded fp32 -> cast fp16 (ACT) -> DMA-transposed (X-bar) to put E on
     partitions.
  2. KpT = wk^T @ K^T  (E_out x S, fp16 in SBUF);  Vp = V @ wv (S x E_out).
  3. Per (head, s-chunk of 128): scoresT = KpT_h_chunk^T... PE matmul
     (lhsT=KpT chunk (64,128), rhs=QpT_h (64,128)) -> PSUM (s-chunk, P).
     exp on ACT with scale=1/8 and per-partition bias folding the key-padding
     mask (+ constant -4 shift for fp16 range safety); output fp16.
  4. AV: lhsT=expT chunk, rhs=Vp chunk head slice -> C (P, E) accumulated in
     one PSUM bank; denominator l = expT^T @ ones accumulated likewise.
  5. C -> fp16, PE-transpose, out = C @ wo, multiply by 1/l (per-partition),
     DMA out. Bias adds (bq/bk/bv/bo) are emitted only when nonzero.
"""

import os
import sys

import numpy as np

if "/opt/trn_rl_repo" not in sys.path:
    sys.path.insert(0, "/opt/trn_rl_repo")

B, P, S, E = 16, 128, 4096, 512
H = 8
D = E // H
LATENT = 128
N_CORES = 8
BPC = B // N_CORES  # batches per core

SC = 128          # attention S-chunk (scoresT partition dim)
NSC = S // SC     # 32
S8 = 512          # projection S-chunk
NS8 = S // S8     # 8
NE = E // 128     # 4 E-chunks

MASK_ON = -4.0    # constant shift keeps exp() in comfortable fp16 range
MASK_OFF = -34.0  # exp(-34+s/8) ~ 1e-14 -> flushes to 0 in fp16


def build_module(n_batches, has_bq, has_bk, has_bv, has_bo,
                 wide_exp=False, vp_on_act=False):
    import concourse.mybir as mybir
    import concourse.tile as tile
    from concourse import bacc
    from concourse.masks import make_identity
    from contextlib import ExitStack

    f32 = mybir.dt.float32
    f16 = mybir.dt.float16
    AF = mybir.ActivationFunctionType

    nc = bacc.Bacc("TRN2", target_bir_lowering=False, debug=False,
                   enable_asserts=False, num_devices=N_CORES)

    q_d = nc.dram_tensor("q", [n_batches, P, E], f32, kind="ExternalInput").ap()
    k_d = nc.dram_tensor("k", [n_batches, S, E], f32, kind="ExternalInput").ap()
    v_d = nc.dram_tensor("v", [n_batches, S, E], f32, kind="ExternalInput").ap()
    mb_d = nc.dram_tensor("maskbias", [n_batches, SC, NSC], f32,
                          kind="ExternalInput").ap()
    wq_d = nc.dram_tensor("wq", [E, E], f32, kind="ExternalInput").ap()
    wk_d = nc.dram_tensor("wk", [E, E], f32, kind="ExternalInput").ap()
    wv_d = nc.dram_tensor("wv", [E, E], f32, kind="ExternalInput").ap()
    wo_d = nc.dram_tensor("wo", [E, LATENT], f32, kind="ExternalInput").ap()
    bq_d = nc.dram_tensor("bq", [E], f32, kind="ExternalInput").ap() if has_bq else None
    bk_d = nc.dram_tensor("bk", [E], f32, kind="ExternalInput").ap() if has_bk else None
    bv_d = nc.dram_tensor("bv", [E], f32, kind="ExternalInput").ap() if has_bv else None
    bo_d = nc.dram_tensor("bo", [LATENT], f32, kind="ExternalInput").ap() if has_bo else None
    out_d = nc.dram_tensor("out", [n_batches, P, LATENT], f32,
                           kind="ExternalOutput").ap()

    with tile.TileContext(nc) as tc, ExitStack() as ctx:
        const_pool = ctx.enter_context(tc.tile_pool(name="const", bufs=1))
        wstage_pool = ctx.enter_context(tc.tile_pool(name="wstage", bufs=2))
        kv_raw_pool = ctx.enter_context(tc.tile_pool(name="kvraw", bufs=6))
        kt_pool = ctx.enter_context(tc.tile_pool(name="ktw", bufs=2))
        vt_pool = ctx.enter_context(tc.tile_pool(name="vtw", bufs=2))
        kpt_pool = ctx.enter_context(tc.tile_pool(name="kptw", bufs=2))
        vp_pool = ctx.enter_context(tc.tile_pool(name="vp", bufs=2))
        qp_pool = ctx.enter_context(tc.tile_pool(name="qp", bufs=2))
        mb_pool = ctx.enter_context(tc.tile_pool(name="mb", bufs=2))
        ex_pool = ctx.enter_context(tc.tile_pool(name="ex", bufs=6))
        fin_pool = ctx.enter_context(tc.tile_pool(name="fin", bufs=2))

        # PSUM budget is 8 banks:
        # tp (2) + proj (2) + sc (2, shared with Q-proj / out-proj) + C (2)
        ps_tp_pool = ctx.enter_context(
            tc.tile_pool(name="pstp", bufs=2, space="PSUM"))
        ps_proj_pool = ctx.enter_context(
            tc.tile_pool(name="psproj", bufs=2, space="PSUM"))
        ps_sc_pool = ctx.enter_context(
            tc.tile_pool(name="pssc", bufs=2, space="PSUM"))
        ps_c_pool = ctx.enter_context(
            tc.tile_pool(name="psc", bufs=1, space="PSUM"))

        # ---- constants & weights (once) ----
        ident32 = const_pool.tile([128, 128], f32, name="ident32")
        make_identity(nc, ident32[:])
        ident16 = const_pool.tile([128, 128], f16, name="ident16")
        make_identity(nc, ident16[:])

        def load_weight_f16(w_ap, ncols, name):
            tiles = []
            for j in range(NE):
                raw = wstage_pool.tile([128, ncols], f32, tag="wraw")
                nc.sync.dma_start(raw[:], w_ap[j * 128:(j + 1) * 128, :])
                w16 = const_pool.tile([128, ncols], f16, name=f"{name}{j}")
                nc.scalar.copy(w16[:], raw[:])
                tiles.append(w16)
            return tiles

        wq_h = load_weight_f16(wq_d, E, "wq")
        wk_h = load_weight_f16(wk_d, E, "wk")
        wv_h = load_weight_f16(wv_d, E, "wv")
        wo_h = load_weight_f16(wo_d, LATENT, "wo")

        ones_row = None
        if has_bv or has_bo:
            ones_row = const_pool.tile([1, 128], f16, name="ones_row")
            nc.vector.memset(ones_row[:], 1.0)

        def load_vec_f16(b_ap, n, name):
            raw = wstage_pool.tile([1, n], f32, tag="braw")
            nc.sync.dma_start(raw[:], b_ap[None, :])
            v16 = const_pool.tile([1, n], f16, name=name)
            nc.scalar.copy(v16[:], raw[:])
            return v16

        bv_row = load_vec_f16(bv_d, E, "bv_row") if has_bv else None
        bo_row = load_vec_f16(bo_d, LATENT, "bo_row") if has_bo else None

        def load_bias_cols(b_ap, name):
            tiles = []
            for j in range(NE):
                t = const_pool.tile([128, 1], f32, name=f"{name}{j}")
                nc.sync.dma_start(t[:], b_ap[j * 128:(j + 1) * 128, None])
                tiles.append(t)
            return tiles

        bq_cols = load_bias_cols(bq_d, "bq") if has_bq else None
        bk_cols = load_bias_cols(bk_d, "bk") if has_bk else None

        for b in range(n_batches):
            mbias = mb_pool.tile([128, NSC], f32, tag="mb")
            nc.sync.dma_start(mbias[:], mb_d[b])

            # ---- Q path: PE-transpose raw fp32 Q, cast on copy-out ----
            qraw = kv_raw_pool.tile([128, E], f32, tag="qraw")
            nc.sync.dma_start(qraw[:], q_d[b])
            qt = []
            for j in range(NE):
                tq = ps_tp_pool.tile([128, 128], f32, tag="tp")
                nc.tensor.transpose(tq[:], qraw[:, j * 128:(j + 1) * 128],
                                    ident32[:])
                qt_j = qp_pool.tile([128, 128], f16, tag=f"qt{j}")
                nc.scalar.copy(qt_j[:], tq[:])
                qt.append(qt_j)
            qpt = []
            for m in range(NE):
                ps = ps_sc_pool.tile([128, 128], f32, tag="s")
                for j in range(NE):
                    nc.tensor.matmul(ps[:], wq_h[j][:, m * 128:(m + 1) * 128],
                                     qt[j][:], start=(j == 0), stop=(j == NE - 1))
                t = qp_pool.tile([128, 128], f16, tag=f"qpt{m}")
                if has_bq:
                    nc.scalar.activation(t[:], ps[:], AF.Identity,
                                         bias=bq_cols[m][:], scale=1.0)
                else:
                    nc.vector.tensor_copy(t[:], ps[:])
                qpt.append(t)

            # ---- C~ accumulators: 4 heads x (64 AV cols + 1 denom col) each ----
            c_ps = [ps_c_pool.tile([128, 4, D + 1], f32, tag=f"c{i}",
                                   name=f"c{b}_{i}")
                    for i in range(2)]

            for s8 in range(NS8):
                # -- load + PE-transpose K,V (fp32), cast fp16 on copy-out --
                ktw = [kt_pool.tile([128, S8], f16, tag=f"kt{j}",
                                    name=f"kt{b}_{s8}_{j}") for j in range(NE)]
                vtw = [vt_pool.tile([128, S8], f16, tag=f"vt{j}",
                                    name=f"vt{b}_{s8}_{j}") for j in range(NE)]
                for i in range(4):
                    s0 = s8 * S8 + i * 128
                    kraw = kv_raw_pool.tile([128, E], f32, tag="kraw")
                    nc.sync.dma_start(kraw[:], k_d[b, s0:s0 + 128, :])
                    vraw = kv_raw_pool.tile([128, E], f32, tag="vraw")
                    nc.sync.dma_start(vraw[:], v_d[b, s0:s0 + 128, :])
                    tpk = ps_tp_pool.tile([128, E], f32, tag="tp")
                    tpv = ps_tp_pool.tile([128, E], f32, tag="tp")
                    for j in range(NE):
                        nc.tensor.transpose(tpk[:, j * 128:(j + 1) * 128],
                                            kraw[:, j * 128:(j + 1) * 128],
                                            ident32[:])
                        nc.tensor.transpose(tpv[:, j * 128:(j + 1) * 128],
                                            vraw[:, j * 128:(j + 1) * 128],
                                            ident32[:])
                    for j in range(NE):
                        nc.scalar.copy(ktw[j][:, i * 128:(i + 1) * 128],
                                       tpk[:, j * 128:(j + 1) * 128])
                        nc.vector.tensor_copy(vtw[j][:, i * 128:(i + 1) * 128],
                                              tpv[:, j * 128:(j + 1) * 128])

                # -- KpT for this S8 window: 4 x (128 E_out, 512 S) --
                kptw = [kpt_pool.tile([128, S8], f16, tag=f"kpt{m}",
                                      name=f"kpt{b}_{s8}_{m}")
                        for m in range(NE)]
                for m in range(NE):
                    ps = ps_proj_pool.tile([128, S8], f32, tag="proj")
                    for j in range(NE):
                        nc.tensor.matmul(ps[:], wk_h[j][:, m * 128:(m + 1) * 128],
                                         ktw[j][:], start=(j == 0),
                                         stop=(j == NE - 1))
                    if has_bk:
                        nc.scalar.activation(kptw[m][:], ps[:], AF.Identity,
                                             bias=bk_cols[m][:], scale=1.0)
                    else:
                        nc.vector.tensor_copy(kptw[m][:], ps[:])

                # -- Vp for the 4 chunks of this S8 window --
                vp = []
                for i in range(4):
                    ps = ps_proj_pool.tile([128, E], f32, tag="proj")
                    for j in range(NE):
                        nc.tensor.matmul(ps[:], vtw[j][:, i * 128:(i + 1) * 128],
                                         wv_h[j][:], start=(j == 0),
                                         stop=(j == NE - 1 and not has_bv))
                    if has_bv:
                        nc.tensor.matmul(ps[:], ones_row[:], bv_row[:],
                                         start=False, stop=True)
                    vpt = vp_pool.tile([128, H, D + 1], f16, tag=f"vp{i}",
                                       name=f"vp{b}_{s8}_{i}")
                    if vp_on_act:
                        nc.scalar.activation(
                            vpt[:, :, 0:D],
                            ps[:].rearrange("p (h x) -> p h x", h=H), AF.Copy)
                    else:
                        nc.vector.tensor_copy(
                            vpt[:, :, 0:D],
                            ps[:].rearrange("p (h x) -> p h x", h=H))
                    nc.vector.memset(vpt[:, :, D:D + 1], 1.0)
                    vp.append(vpt)

                # -- attention for the 4 chunks of this S8 window --
                for i in range(4):
                    c = s8 * 4 + i
                    nheads = 4 if wide_exp else 1  # wide_exp may be 'contig_ex'
                    for g in range(H // nheads):
                        s_ps = ps_sc_pool.tile([128, 128 * nheads], f32,
                                               tag="s")
                        for hh in range(nheads):
                            h = nheads * g + hh
                            t, off = h // 2, (h % 2) * 64
                            # one zero-region group per bank: start on the
                            # first matmul only, stop on the last
                            nc.tensor.matmul(
                                s_ps[:, hh * 128:(hh + 1) * 128],
                                kptw[t][off:off + 64, i * 128:(i + 1) * 128],
                                qpt[t][off:off + 64, :],
                                start=True, stop=True)
                        if wide_exp == "contig_ex":
                            exs = []
                            for hh in range(nheads):
                                exn = ex_pool.tile([128, 128], f16, tag="ex")
                                nc.scalar.activation(
                                    exn[:], s_ps[:, hh * 128:(hh + 1) * 128],
                                    AF.Exp, bias=mbias[:, c:c + 1], scale=0.125)
                                exs.append(exn)
                            get_ex = lambda hh: exs[hh][:]
                        else:
                            ex = ex_pool.tile([128, 128 * nheads], f16,
                                              tag="ex")
                            nc.scalar.activation(ex[:], s_ps[:], AF.Exp,
                                                 bias=mbias[:, c:c + 1],
                                                 scale=0.125)
                            get_ex = lambda hh: ex[:, hh * 128:(hh + 1) * 128]
                        for hh in range(nheads):
                            h = nheads * g + hh
                            nc.tensor.matmul(
                                c_ps[h // 4][:, h % 4, :],
                                get_ex(hh),
                                vp[i][:, h, :],
                                start=(c == 0 and h % 4 == 0),
                                stop=(c == NSC - 1 and h % 4 == 3))

            # ---- normalize per head, then output projection ----
            rl_sb = fin_pool.tile([128, H], f32, tag="rl_sb")
            for h in range(H):
                nc.vector.reciprocal(rl_sb[:, h:h + 1],
                                     c_ps[h // 4][:, h % 4, D:D + 1])
            csb = fin_pool.tile([128, E], f16, tag="csb")
            for h in range(H):
                nc.vector.tensor_scalar_mul(csb[:, h * D:(h + 1) * D],
                                            c_ps[h // 4][:, h % 4, 0:D],
                                            rl_sb[:, h:h + 1])
            ct = []
            for j in range(NE):
                ct_ps = ps_sc_pool.tile([128, 128], f16, tag="s")
                nc.tensor.transpose(ct_ps[:], csb[:, j * 128:(j + 1) * 128],
                                    ident16[:])
                ct_j = fin_pool.tile([128, 128], f16, tag=f"ct{j}")
                nc.vector.tensor_copy(ct_j[:], ct_ps[:])
                ct.append(ct_j)
            o_ps = ps_sc_pool.tile([128, LATENT], f32, tag="s")
            for j in range(NE):
                nc.tensor.matmul(o_ps[:], ct[j][:], wo_h[j][:],
                                 start=(j == 0), stop=(j == NE - 1 and not has_bo))
            if has_bo:
                nc.tensor.matmul(o_ps[:], ones_row[:], bo_row[:],
                                 start=False, stop=True)
            osb = fin_pool.tile([128, LATENT], f32, tag="osb")
            nc.vector.tensor_copy(osb[:], o_ps[:])
            nc.sync.dma_start(out_d[b], osb[:])

    nc.compile()
    return nc


_module_cache = {}


def _get_module(n_batches, flags):
    key = (n_batches, flags)
    if key not in _module_cache:
        _module_cache[key] = build_module(n_batches, *flags)
    return _module_cache[key]


def make_maskbias(mask):
    # (B, S) bool -> (B, SC, NSC) f32 with [b, p, c] = bias for s = c*128+p
    mb = np.where(mask, MASK_ON, MASK_OFF).astype(np.float32)
    return np.ascontiguousarray(mb.reshape(-1, NSC, SC).transpose(0, 2, 1))


def make_in_maps(Q, K, V, mask, wq, bq, wk, bk, wv, bv, wo, bo, n_cores, bpc):
    flags = (bool(np.any(bq)), bool(np.any(bk)),
             bool(np.any(bv)), bool(np.any(bo)))
    mb = make_maskbias(np.asarray(mask))
    f = np.ascontiguousarray
    in_maps = []
    for i in range(n_cores):
        sl = slice(i * bpc, (i + 1) * bpc)
        m = {"q": f(Q[sl]), "k": f(K[sl]), "v": f(V[sl]), "maskbias": f(mb[sl]),
             "wq": f(wq), "wk": f(wk), "wv": f(wv), "wo": f(wo)}
        if flags[0]:
            m["bq"] = f(bq)
        if flags[1]:
            m["bk"] = f(bk)
        if flags[2]:
            m["bv"] = f(bv)
        if flags[3]:
            m["bo"] = f(bo)
        in_maps.append(m)
    return in_maps, flags


def kernel(**inputs):
    from concourse.bass_utils import run_bass_kernel_spmd

    args = {k: np.asarray(v) for k, v in inputs.items()}
    in_maps, flags = make_in_maps(
        args["Q"].astype(np.float32), args["K"].astype(np.float32),
        args["V"].astype(np.float32), args["mask"],
        args["wq"].astype(np.float32), args["bq"].astype(np.float32),
        args["wk"].astype(np.float32), args["bk"].astype(np.float32),
        args["wv"].astype(np.float32), args["bv"].astype(np.float32),
        args["wo"].astype(np.float32), args["bo"].astype(np.float32),
        N_CORES, BPC)
    nc = _get_module(BPC, flags)
    res = run_bass_kernel_spmd(nc, in_maps, core_ids=list(range(N_CORES)))
    kernel.last_results = res
    if res.exec_time_ns is not None:
        print(f"HW exec time: {res.exec_time_ns} ns")
    out = np.concatenate([r["out"] for r in res.results], axis=0)
    return out.astype(np.float32)


# revision 19
# speedup vs baseline: 2.6855x; 1.0390x over previous
"""FLDAttention Trainium2 kernel.

B=16, P=128, S=4096, E=512, H=8, D=64, LATENT=128.
Sharding: data-parallel over B across 8 cores (2 batches/core), no collectives.

Per-batch on-device pipeline (all matmul operands fp16, fp32 PSUM accum):
  1. K,V loaded fp32 -> cast fp16 (ACT) -> DMA-transposed (X-bar) to put E on
     partitions.
  2. KpT = wk^T @ K^T  (E_out x S, fp16 in SBUF);  Vp = V @ wv (S x E_out).
  3. Per (head, s-chunk of 128): scoresT = KpT_h_chunk^T... PE matmul
     (lhsT=KpT chunk (64,128), rhs=QpT_h (64,128)) -> PSUM (s-chunk, P).
     exp on ACT with scale=1/8 and per-partition bias folding the key-padding
     mask (+ constant -4 shift for fp16 range safety); output fp16.
  4. AV: lhsT=expT chunk, rhs=Vp chunk head slice -> C (P, E) accumulated in
     one PSUM bank; denominator l = expT^T @ ones accumulated likewise.
  5. C -> fp16, PE-transpose, out = C @ wo, multiply by 1/l (per-partition),
     DMA out. Bias adds (bq/bk/bv/bo) are emitted only when nonzero.
"""

import os
import sys

import numpy as np

if "/opt/trn_rl_repo" not in sys.path:
    sys.path.insert(0, "/opt/trn_rl_repo")

B, P, S, E = 16, 128, 4096, 512
H = 8
D = E // H
LATENT = 128
N_CORES = 8
BPC = B // N_CORES  # batches per core

SC = 128          # attention S-chunk (scoresT partition dim)
NSC = S // SC     # 32
S8 = 512          # projection S-chunk
NS8 = S // S8     # 8
NE = E // 128     # 4 E-chunks

MASK_ON = -4.0    # constant shift keeps exp() in comfortable fp16 range
MASK_OFF = -34.0  # exp(-34+s/8) ~ 1e-14 -> flushes to 0 in fp16


def build_module(n_batches, has_bq, has_bk, has_bv, has_bo,
                 wide_exp=False, vp_on_act=False):
    import concourse.mybir as mybir
    import concourse.tile as tile
    from concourse import bacc
    from concourse.masks import make_identity
    from contextlib import ExitStack

    f32 = mybir.dt.float32
    f16 = mybir.dt.float16
    AF = mybir.ActivationFunctionType

    nc = bacc.Bacc("TRN2", target_bir_lowering=False, debug=False,
                   enable_asserts=False, num_devices=N_CORES)

    q_d = nc.dram_tensor("q", [n_batches, P, E], f32, kind="ExternalInput").ap()
    k_d = nc.dram_tensor("k", [n_batches, S, E], f32, kind="ExternalInput").ap()
    v_d = nc.dram_tensor("v", [n_batches, S, E], f32, kind="ExternalInput").ap()
    mb_d = nc.dram_tensor("maskbias", [n_batches, SC, NSC], f32,
                          kind="ExternalInput").ap()
    wq_d = nc.dram_tensor("wq", [E, E], f32, kind="ExternalInput").ap()
    wk_d = nc.dram_tensor("wk", [E, E], f32, kind="ExternalInput").ap()
    wv_d = nc.dram_tensor("wv", [E, E], f32, kind="ExternalInput").ap()
    wo_d = nc.dram_tensor("wo", [E, LATENT], f32, kind="ExternalInput").ap()
    bq_d = nc.dram_tensor("bq", [E], f32, kind="ExternalInput").ap() if has_bq else None
    bk_d = nc.dram_tensor("bk", [E], f32, kind="ExternalInput").ap() if has_bk else None
    bv_d = nc.dram_tensor("bv", [E], f32, kind="ExternalInput").ap() if has_bv else None
    bo_d = nc.dram_tensor("bo", [LATENT], f32, kind="ExternalInput").ap() if has_bo else None
    out_d = nc.dram_tensor("out", [n_batches, P, LATENT], f32,
                           kind="ExternalOutput").ap()

    with tile.TileContext(nc) as tc, ExitStack() as ctx:
        const_pool = ctx.enter_context(tc.tile_pool(name="const", bufs=1))
        wstage_pool = ctx.enter_context(tc.tile_pool(name="wstage", bufs=2))
        kv_raw_pool = ctx.enter_context(tc.tile_pool(name="kvraw", bufs=6))
        kt_pool = ctx.enter_context(tc.tile_pool(name="ktw", bufs=2))
        vt_pool = ctx.enter_context(tc.tile_pool(name="vtw", bufs=2))
        kpt_pool = ctx.enter_context(tc.tile_pool(name="kptw", bufs=2))
        vp_pool = ctx.enter_context(tc.tile_pool(name="vp", bufs=2))
        qp_pool = ctx.enter_context(tc.tile_pool(name="qp", bufs=2))
        mb_pool = ctx.enter_context(tc.tile_pool(name="mb", bufs=2))
        ex_pool = ctx.enter_context(tc.tile_pool(name="ex", bufs=6))
        fin_pool = ctx.enter_context(tc.tile_pool(name="fin", bufs=2))

        # PSUM budget is 8 banks:
        # tp (2) + proj (2) + sc (2, shared with Q-proj / out-proj) + C (2)
        ps_tp_pool = ctx.enter_context(
            tc.tile_pool(name="pstp", bufs=2, space="PSUM"))
        ps_proj_pool = ctx.enter_context(
            tc.tile_pool(name="psproj", bufs=2, space="PSUM"))
        ps_sc_pool = ctx.enter_context(
            tc.tile_pool(name="pssc", bufs=2, space="PSUM"))
        ps_c_pool = ctx.enter_context(
            tc.tile_pool(name="psc", bufs=1, space="PSUM"))

        # ---- constants & weights (once) ----
        ident32 = const_pool.tile([128, 128], f32, name="ident32")
        make_identity(nc, ident32[:])
        ident16 = const_pool.tile([128, 128], f16, name="ident16")
        make_identity(nc, ident16[:])

        def load_weight_f16(w_ap, ncols, name):
            tiles = []
            for j in range(NE):
                raw = wstage_pool.tile([128, ncols], f32, tag="wraw")
                nc.sync.dma_start(raw[:], w_ap[j * 128:(j + 1) * 128, :])
                w16 = const_pool.tile([128, ncols], f16, name=f"{name}{j}")
                nc.scalar.copy(w16[:], raw[:])
                tiles.append(w16)
            return tiles

        wq_h = load_weight_f16(wq_d, E, "wq")
        wk_h = load_weight_f16(wk_d, E, "wk")
        wv_h = load_weight_f16(wv_d, E, "wv")
        wo_h = load_weight_f16(wo_d, LATENT, "wo")

        ones_row = None
        if has_bv or has_bo:
            ones_row = const_pool.tile([1, 128], f16, name="ones_row")
            nc.vector.memset(ones_row[:], 1.0)

        def load_vec_f16(b_ap, n, name):
            raw = wstage_pool.tile([1, n], f32, tag="braw")
            nc.sync.dma_start(raw[:], b_ap[None, :])
            v16 = const_pool.tile([1, n], f16, name=name)
            nc.scalar.copy(v16[:], raw[:])
            return v16

        bv_row = load_vec_f16(bv_d, E, "bv_row") if has_bv else None
        bo_row = load_vec_f16(bo_d, LATENT, "bo_row") if has_bo else None

        def load_bias_cols(b_ap, name):
            tiles = []
            for j in range(NE):
                t = const_pool.tile([128, 1], f32, name=f"{name}{j}")
                nc.sync.dma_start(t[:], b_ap[j * 128:(j + 1) * 128, None])
                tiles.append(t)
            return tiles

        bq_cols = load_bias_cols(bq_d, "bq") if has_bq else None
        bk_cols = load_bias_cols(bk_d, "bk") if has_bk else None

        for b in range(n_batches):
            mbias = mb_pool.tile([128, NSC], f32, tag="mb")
            nc.sync.dma_start(mbias[:], mb_d[b])

            # ---- Q path: PE-transpose raw fp32 Q, cast on copy-out ----
            qraw = kv_raw_pool.tile([128, E], f32, tag="qraw")
            nc.sync.dma_start(qraw[:], q_d[b])
            qt = []
            for j in range(NE):
                tq = ps_tp_pool.tile([128, 128], f32, tag="tp")
                nc.tensor.transpose(tq[:], qraw[:, j * 128:(j + 1) * 128],
                                    ident32[:])
                qt_j = qp_pool.tile([128, 128], f16, tag=f"qt{j}")
                nc.scalar.copy(qt_j[:], tq[:])
                qt.append(qt_j)
            qpt = []
            for m in range(NE):
                ps = ps_sc_pool.tile([128, 128], f32, tag="s")
                for j in range(NE):
                    nc.tensor.matmul(ps[:], wq_h[j][:, m * 128:(m + 1) * 128],
                                     qt[j][:], start=(j == 0), stop=(j == NE - 1))
                t = qp_pool.tile([128, 128], f16, tag=f"qpt{m}")
                if has_bq:
                    nc.scalar.activation(t[:], ps[:], AF.Identity,
                                         bias=bq_cols[m][:], scale=1.0)
                else:
                    nc.vector.tensor_copy(t[:], ps[:])
                qpt.append(t)

            # ---- C~ accumulators: 4 heads x (64 AV cols + 1 denom col) each ----
            c_ps = [ps_c_pool.tile([128, 4, D + 1], f32, tag=f"c{i}",
                                   name=f"c{b}_{i}")
                    for i in range(2)]

            def stage_a(s8):
                # load + PE-transpose K,V (fp32), cast fp16 on copy-out
                ktw = [kt_pool.tile([128, S8], f16, tag=f"kt{j}",
                                    name=f"kt{b}_{s8}_{j}") for j in range(NE)]
                vtw = [vt_pool.tile([128, S8], f16, tag=f"vt{j}",
                                    name=f"vt{b}_{s8}_{j}") for j in range(NE)]
                for i in range(4):
                    s0 = s8 * S8 + i * 128
                    kraw = kv_raw_pool.tile([128, E], f32, tag="kraw")
                    nc.sync.dma_start(kraw[:], k_d[b, s0:s0 + 128, :])
                    vraw = kv_raw_pool.tile([128, E], f32, tag="vraw")
                    nc.sync.dma_start(vraw[:], v_d[b, s0:s0 + 128, :])
                    tpk = ps_tp_pool.tile([128, E], f32, tag="tp")
                    tpv = ps_tp_pool.tile([128, E], f32, tag="tp")
                    for j in range(NE):
                        nc.tensor.transpose(tpk[:, j * 128:(j + 1) * 128],
                                            kraw[:, j * 128:(j + 1) * 128],
                                            ident32[:])
                        nc.tensor.transpose(tpv[:, j * 128:(j + 1) * 128],
                                            vraw[:, j * 128:(j + 1) * 128],
                                            ident32[:])
                    for j in range(NE):
                        nc.scalar.copy(ktw[j][:, i * 128:(i + 1) * 128],
                                       tpk[:, j * 128:(j + 1) * 128])
                        nc.vector.tensor_copy(vtw[j][:, i * 128:(i + 1) * 128],
                                              tpv[:, j * 128:(j + 1) * 128])

                # -- KpT for this S8 window: 4 x (128 E_out, 512 S) --
                kptw = [kpt_pool.tile([128, S8], f16, tag=f"kpt{m}",
                                      name=f"kpt{b}_{s8}_{m}")
                        for m in range(NE)]
                for m in range(NE):
                    ps = ps_proj_pool.tile([128, S8], f32, tag="proj")
                    for j in range(NE):
                        nc.tensor.matmul(ps[:], wk_h[j][:, m * 128:(m + 1) * 128],
                                         ktw[j][:], start=(j == 0),
                                         stop=(j == NE - 1))
                    if has_bk:
                        nc.scalar.activation(kptw[m][:], ps[:], AF.Identity,
                                             bias=bk_cols[m][:], scale=1.0)
                    else:
                        nc.vector.tensor_copy(kptw[m][:], ps[:])

                # -- Vp for the 4 chunks of this S8 window --
                vp = []
                for i in range(4):
                    ps = ps_proj_pool.tile([128, E], f32, tag="proj")
                    for j in range(NE):
                        nc.tensor.matmul(ps[:], vtw[j][:, i * 128:(i + 1) * 128],
                                         wv_h[j][:], start=(j == 0),
                                         stop=(j == NE - 1 and not has_bv))
                    if has_bv:
                        nc.tensor.matmul(ps[:], ones_row[:], bv_row[:],
                                         start=False, stop=True)
                    vpt = vp_pool.tile([128, H, D + 1], f16, tag=f"vp{i}",
                                       name=f"vp{b}_{s8}_{i}")
                    if vp_on_act:
                        nc.scalar.activation(
                            vpt[:, :, 0:D],
                            ps[:].rearrange("p (h x) -> p h x", h=H), AF.Copy)
                    else:
                        nc.vector.tensor_copy(
                            vpt[:, :, 0:D],
                            ps[:].rearrange("p (h x) -> p h x", h=H))
                    nc.vector.memset(vpt[:, :, D:D + 1], 1.0)
                    vp.append(vpt)

                return kptw, vp

            def stage_b(s8, kptw, vp):
                # attention for the 4 chunks of this S8 window
                for i in range(4):
                    c = s8 * 4 + i
                    nheads = 4 if wide_exp else 1  # wide_exp may be 'contig_ex'
                    for g in range(H // nheads):
                        s_ps = ps_sc_pool.tile([128, 128 * nheads], f32,
                                               tag="s")
                        for hh in range(nheads):
                            h = nheads * g + hh
                            t, off = h // 2, (h % 2) * 64
                            # one zero-region group per bank: start on the
                            # first matmul only, stop on the last
                            nc.tensor.matmul(
                                s_ps[:, hh * 128:(hh + 1) * 128],
                                kptw[t][off:off + 64, i * 128:(i + 1) * 128],
                                qpt[t][off:off + 64, :],
                                start=True, stop=True)
                        if wide_exp == "contig_ex":
                            exs = []
                            for hh in range(nheads):
                                exn = ex_pool.tile([128, 128], f16, tag="ex")
                                nc.scalar.activation(
                                    exn[:], s_ps[:, hh * 128:(hh + 1) * 128],
                                    AF.Exp, bias=mbias[:, c:c + 1], scale=0.125)
                                exs.append(exn)
                            get_ex = lambda hh: exs[hh][:]
                        else:
                            ex = ex_pool.tile([128, 128 * nheads], f16,
                                              tag="ex")
                            nc.scalar.activation(ex[:], s_ps[:], AF.Exp,
                                                 bias=mbias[:, c:c + 1],
                                                 scale=0.125)
                            get_ex = lambda hh: ex[:, hh * 128:(hh + 1) * 128]
                        for hh in range(nheads):
                            h = nheads * g + hh
                            nc.tensor.matmul(
                                c_ps[h // 4][:, h % 4, :],
                                get_ex(hh),
                                vp[i][:, h, :],
                                start=(c == 0 and h % 4 == 0),
                                stop=(c == NSC - 1 and h % 4 == 3))

            prev = None
            for s8 in range(NS8):
                cur = stage_a(s8)
                if prev is not None:
                    stage_b(s8 - 1, *prev)
                prev = cur
            stage_b(NS8 - 1, *prev)

            # ---- normalize per head, then output projection ----
            rl_sb = fin_pool.tile([128, H], f32, tag="rl_sb")
            for h in range(H):
                nc.vector.reciprocal(rl_sb[:, h:h + 1],
                                     c_ps[h // 4][:, h % 4, D:D + 1])
            csb = fin_pool.tile([128, E], f16, tag="csb")
            for h in range(H):
                nc.vector.tensor_scalar_mul(csb[:, h * D:(h + 1) * D],
                                            c_ps[h // 4][:, h % 4, 0:D],
                                            rl_sb[:, h:h + 1])
            ct = []
            for j in range(NE):
                ct_ps = ps_sc_pool.tile([128, 128], f16, tag="s")
                nc.tensor.transpose(ct_ps[:], csb[:, j * 128:(j + 1) * 128],
                                    ident16[:])
                ct_j = fin_pool.tile([128, 128], f16, tag=f"ct{j}")
                nc.vector.tensor_copy(ct_j[:], ct_ps[:])
                ct.append(ct_j)
            o_ps = ps_sc_pool.tile([128, LATENT], f32, tag="s")
            for j in range(NE):
                nc.tensor.matmul(o_ps[:], ct[j][:], wo_h[j][:],
                                 start=(j == 0), stop=(j == NE - 1 and not has_bo))
            if has_bo:
                nc.tensor.matmul(o_ps[:], ones_row[:], bo_row[:],
                                 start=False, stop=True)
            osb = fin_pool.tile([128, LATENT], f32, tag="osb")
            nc.vector.tensor_copy(osb[:], o_ps[:])
            nc.sync.dma_start(out_d[b], osb[:])

    nc.compile()
    return nc


_module_cache = {}


def _get_module(n_batches, flags):
    key = (n_batches, flags)
    if key not in _module_cache:
        _module_cache[key] = build_module(n_batches, *flags)
    return _module_cache[key]


def make_maskbias(mask):
    # (B, S) bool -> (B, SC, NSC) f32 with [b, p, c] = bias for s = c*128+p
    mb = np.where(mask, MASK_ON, MASK_OFF).astype(np.float32)
    return np.ascontiguousarray(mb.reshape(-1, NSC, SC).transpose(0, 2, 1))


def make_in_maps(Q, K, V, mask, wq, bq, wk, bk, wv, bv, wo, bo, n_cores, bpc):
    flags = (bool(np.any(bq)), bool(np.any(bk)),
             bool(np.any(bv)), bool(np.any(bo)))
    mb = make_maskbias(np.asarray(mask))
    f = np.ascontiguousarray
    in_maps = []
    for i in range(n_cores):
        sl = slice(i * bpc, (i + 1) * bpc)
        m = {"q": f(Q[sl]), "k": f(K[sl]), "v": f(V[sl]), "maskbias": f(mb[sl]),
             "wq": f(wq), "wk": f(wk), "wv": f(wv), "wo": f(wo)}
        if flags[0]:
            m["bq"] = f(bq)
        if flags[1]:
            m["bk"] = f(bk)
        if flags[2]:
            m["bv"] = f(bv)
        if flags[3]:
            m["bo"] = f(bo)
        in_maps.append(m)
    return in_maps, flags


def kernel(**inputs):
    from concourse.bass_utils import run_bass_kernel_spmd

    args = {k: np.asarray(v) for k, v in inputs.items()}
    in_maps, flags = make_in_maps(
        args["Q"].astype(np.float32), args["K"].astype(np.float32),
        args["V"].astype(np.float32), args["mask"],
        args["wq"].astype(np.float32), args["bq"].astype(np.float32),
        args["wk"].astype(np.float32), args["bk"].astype(np.float32),
        args["wv"].astype(np.float32), args["bv"].astype(np.float32),
        args["wo"].astype(np.float32), args["bo"].astype(np.float32),
        N_CORES, BPC)
    nc = _get_module(BPC, flags)
    res = run_bass_kernel_spmd(nc, in_maps, core_ids=list(range(N_CORES)))
    kernel.last_results = res
    if res.exec_time_ns is not None:
        print(f"HW exec time: {res.exec_time_ns} ns")
    out = np.concatenate([r["out"] for r in res.results], axis=0)
    return out.astype(np.float32)
